# revision 1
# baseline (speedup 1.0000x reference)
"""2-layer GAT (PyG GATConv semantics) on 8 Trainium2 NeuronCores via Bass.

Contract: kernel(**inputs) takes the FULL unsharded inputs of
reference.setup_inputs() and returns the FULL [100000, 32] float32 output.

Strategy (edge/dst parallel, no collectives):
  * Host: add self-loops, sort nodes by in-degree (desc), cut the sorted node
    list into 128-node tiles, deal tiles round-robin onto the 8 cores, and
    build per-core ELL-style gather-index arrays (slot k=0 = self-loop,
    padding slots point at a dummy table row whose a_src = -87 so its
    exp-weight underflows to ~0).  Each dst node's whole in-edge segment
    lives on one core, so softmax needs no cross-core reduction.
  * Launch 1 (conv1): every core runs the identical SPMD program:
      - replicated GEMM  T1[q] = x_perm[q] @ W1ext,
        W1ext = [W1 | W1@att_src per head | W1@att_dst per head] -> 68 cols
      - per dst-tile: gather all slot rows with per-partition indirect DMAs,
        alpha = a_s[src] + a_d[dst];  p = max(exp(alpha), exp(0.2*alpha))
        (identical to exp(leaky_relu(alpha)));  denominators from the exp's
        accumulate output;  msg accumulate + normalize + ReLU on DVE/ACT.
    Output: per-core [tpc*128, 64] rows in permuted order.
  * Host: reassemble, transpose; Launch 2 (conv2) same shape with 34-col
    table; host inverse-permutes the result.

No segment-max subtraction: |alpha| <~ 8 here, exp is safe in f32, and
softmax is shift-invariant, so results match the reference to fp32 roundoff.

Measured (HW, 8 axon-tunneled trn2 cores, differential wall-clock):
  * full output relerr vs fp32 reference: 2.65e-06
  * edge phase: ~2.43 ms/layer, bound by ~1.5 us per [P,1] indirect-DMA
    gather call (SWDGE Q7 emission serializes on the Pool engine), NOT by
    data (DMA-engine floor is ~0.33 ms/layer at 272 B/row).
Next steps (not landed — see notes): amortize the per-call cost with
InstDMAGatherAnt (one call per few thousand rows). Constraints mapped so
far: int16 indices force <=32768-row table blocks (residue-mod-4 row
classes with per-class base offsets keep the slot grid exact); mid-stream
negative indices are ZERO-FILLED on HW (verified), so per-class passes
must land in separate buffers merged with adds (zeros are the additive
identity). The NRT crashes seen with repeated dma_gather calls were
root-caused and VERIFIED on HW: single_packet=True with >64 descriptors
violates the SDMA packet limit; with single_packet=False, 8 back-to-back
1024-index gathers run correctly (err 0.0). So the full redesign is:
per ~4-dst-tile group x 4 residue classes, one dma_gather
(single_packet=False, num_idxs~2048, elem 512B f32 rows padded to 128
cols, per-class in_ap base offset, idx/4 as int16) into 4 class buffers,
3 DVE merge-adds, then the existing per-tile compute. Expected:
~0.4-0.7 ms/layer vs the current 2.43 ms/layer.
"""

import os
import sys

os.environ.setdefault("JAX_PLATFORMS", "axon")
if "/opt/trn_rl_repo" not in sys.path:
    sys.path.insert(0, "/opt/trn_rl_repo")

from dataclasses import dataclass, field

import numpy as np

import concourse.bass as bass
import concourse.mybir as mybir
import concourse.tile as tile
from concourse import bacc

F32 = mybir.dt.float32
I32 = mybir.dt.int32

P = 128
DUMMY_AS = -87.0  # inside ScalarE Exp valid range; exp(-87) ~ 6e-38

# problem constants (hardcoded per the harness contract)
N_NODES = 100000
IN_CH = 128
HID = 32
HEADS1 = 2
OUT_CH = 32
NCORES = 8
NEG_SLOPE = 0.2


@dataclass
class Cfg:
    n: int = N_NODES
    in_ch: int = IN_CH
    hid: int = HID
    heads: int = HEADS1
    out_ch: int = OUT_CH
    ncores: int = NCORES
    gemm_chunk: int = 512
    neg_slope: float = NEG_SLOPE
    npad: int = 0
    ntiles_g: int = 0
    tpc: int = 0
    k_sched: list = field(default_factory=list)

    @property
    def d1(self):
        return self.heads * self.hid + 2 * self.heads  # 68

    @property
    def d2(self):
        return self.out_ch + 2  # 34

    @property
    def sk(self):
        return int(sum(self.k_sched))


# ----------------------------------------------------------------- host side


def preprocess(cfg: Cfg, edge_index: np.ndarray):
    """Permutation + per-core ELL gather-index arrays + shared K schedule."""
    n, nc_ = cfg.n, cfg.ncores
    src = np.asarray(edge_index[0], dtype=np.int64)
    dst = np.asarray(edge_index[1], dtype=np.int64)

    deg = np.bincount(dst, minlength=n).astype(np.int64) + 1  # + self-loop

    ntiles_real = -(-n // P)
    ntiles_g = -(-ntiles_real // nc_) * nc_
    npad = ntiles_g * P

    perm = np.argsort(-deg, kind="stable")  # position -> original id
    pos_of = np.empty(n, dtype=np.int64)
    pos_of[perm] = np.arange(n)

    deg_sorted = np.concatenate([deg[perm], np.ones(npad - n, dtype=np.int64)])

    tpc = ntiles_g // nc_
    # local tile j spans global tiles j*nc_ .. j*nc_+nc_-1; degrees are
    # non-increasing so the first node of tile j*nc_ has the group max.
    k_sched = [int(deg_sorted[(j * nc_) * P]) for j in range(tpc)]
    sk = int(sum(k_sched))

    order = np.argsort(pos_of[dst], kind="stable")
    src_by_dstpos = pos_of[src[order]].astype(np.int64)
    dstpos_sorted = pos_of[dst[order]]
    starts = np.searchsorted(dstpos_sorted, np.arange(npad))

    dummy = npad
    kmax = int(deg_sorted.max())
    ell = np.full((npad, kmax), dummy, dtype=np.int32)
    ell[:, 0] = np.arange(npad)  # self-loop slot
    col = 1 + np.arange(len(order)) - starts[dstpos_sorted]
    ell[dstpos_sorted, col] = src_by_dstpos

    idx_arrays = []
    offs = np.concatenate([[0], np.cumsum(k_sched)]).astype(np.int64)
    for c in range(nc_):
        arr = np.full((P, sk), dummy, dtype=np.int32)
        for j in range(tpc):
            base = (j * nc_ + c) * P
            kj = k_sched[j]
            arr[:, offs[j] : offs[j] + kj] = ell[base : base + P, :kj]
        idx_arrays.append(arr)

    cfg.npad = npad
    cfg.ntiles_g = ntiles_g
    cfg.tpc = tpc
    cfg.k_sched = k_sched
    return perm, idx_arrays


def make_wext1(W1, att_src1, att_dst1, heads, hid):
    IN = W1.shape[0]
    w = np.zeros((IN, heads * hid + 2 * heads), dtype=np.float32)
    w[:, : heads * hid] = W1
    for h in range(heads):
        w[:, heads * hid + h] = W1[:, h * hid : (h + 1) * hid] @ att_src1[h]
        w[:, heads * hid + heads + h] = W1[:, h * hid : (h + 1) * hid] @ att_dst1[h]
    return w


def make_wext2(W2, att_src2, att_dst2, out_ch):
    IN = W2.shape[0]
    w = np.zeros((IN, out_ch + 2), dtype=np.float32)
    w[:, :out_ch] = W2
    w[:, out_ch] = W2 @ att_src2[0]
    w[:, out_ch + 1] = W2 @ att_dst2[0]
    return w


# ------------------------------------------------------------- kernel builder


def _build_common(cfg: Cfg, layer: int):
    heads = cfg.heads if layer == 1 else 1
    ch = cfg.hid if layer == 1 else cfg.out_ch
    d = cfg.d1 if layer == 1 else cfg.d2
    kin = cfg.in_ch if layer == 1 else cfg.heads * cfg.hid
    outw = heads * ch
    hcols = heads * ch
    npad, tpc = cfg.npad, cfg.tpc
    CH = cfg.gemm_chunk
    assert npad % CH == 0 and CH % P == 0

    # Bacc (not raw Bass): its compile() pass splits multi-waits into event
    # semaphores and moves matmul waits to ldweights — walrus ISA structs only
    # fit one sync wait per instruction.
    nc = bacc.Bacc(None, target_bir_lowering=False)
    xt = nc.declare_dram_parameter("xt", [kin, npad], F32, isOutput=False)
    wext = nc.declare_dram_parameter("wext", [kin, d], F32, isOutput=False)
    idx = nc.declare_dram_parameter("idx", [P, cfg.sk], I32, isOutput=False)
    outl = nc.declare_dram_parameter("outl", [tpc * P, outw], F32, isOutput=True)
    t_tab = nc.dram_tensor("t_tab", [npad + 1, d], F32)

    with tile.TileContext(nc) as tc:
        with (
            tc.tile_pool(name="singles", bufs=1) as singles,
            tc.tile_pool(name="gchunk", bufs=3) as gchunk,
            tc.tile_pool(name="hout", bufs=4) as hout,
            tc.tile_pool(name="psum", bufs=4, space="PSUM") as psum,
            tc.tile_pool(name="gbuf", bufs=3) as gbufp,
            tc.tile_pool(name="small", bufs=4) as small,
            tc.tile_pool(name="mbuf", bufs=3) as mbufp,
            tc.tile_pool(name="obuf", bufs=3) as obufp,
        ):
            # ---- constants
            w_s = singles.tile([kin, d], F32)
            nc.sync.dma_start(out=w_s[:, :], in_=wext[:, :])
            idx_s = singles.tile([P, cfg.sk], I32)
            nc.sync.dma_start(out=idx_s[:, :], in_=idx[:, :])
            cw = singles.tile([1, d], F32)
            nc.vector.memset(cw[:, :], 0.0)
            nc.vector.memset(cw[0:1, hcols : hcols + heads], DUMMY_AS)
            nc.sync.dma_start(out=t_tab[npad : npad + 1, :], in_=cw[0:1, :])

            # ---- phase 1: table GEMM  t_tab[q] = x[q] @ wext
            for ci in range(npad // CH):
                xt_t = gchunk.tile([kin, CH], F32)
                nc.sync.dma_start(out=xt_t[:, :], in_=xt[:, ci * CH : (ci + 1) * CH])
                for s in range(CH // P):
                    ps = psum.tile([P, d], F32)
                    nc.tensor.matmul(
                        out=ps[:, :],
                        lhsT=xt_t[:, s * P : (s + 1) * P],
                        rhs=w_s[:, :],
                        start=True,
                        stop=True,
                    )
                    ht = hout.tile([P, d], F32)
                    nc.vector.tensor_copy(out=ht[:, :], in_=ps[:, :])
                    r0 = ci * CH + s * P
                    nc.sync.dma_start(out=t_tab[r0 : r0 + P, :], in_=ht[:, :])

            tc.strict_bb_all_engine_barrier()

            # ---- phase 2: per-dst-tile gather + softmax + accumulate
            off = 0
            for j in range(tpc):
                K = cfg.k_sched[j]
                g = gbufp.tile([P, K, d], F32, tag="g")
                # one [P,1] indirect gather per slot column (walrus mis-lowers
                # multi-index offset APs; per-partition single-index is the
                # production-proven form)
                for k in range(K):
                    nc.gpsimd.indirect_dma_start(
                        out=g[:, k, :],
                        out_offset=None,
                        in_=t_tab[:, :],
                        in_offset=bass.IndirectOffsetOnAxis(
                            ap=idx_s[:, off + k : off + k + 1], axis=0
                        ),
                    )

                ebuf1 = small.tile([P, heads, K], F32, tag="e1")
                ebuf2 = small.tile([P, heads, K], F32, tag="e2")
                pbuf = small.tile([P, heads, K], F32, tag="p")
                ybuf = small.tile([P, heads, K], F32, tag="y")
                dnm = small.tile([P, heads], F32, tag="d")
                rcp = small.tile([P, heads], F32, tag="r")

                for h in range(heads):
                    # alpha = a_s[src] + a_d[dst]; a_d from the self-loop row
                    nc.vector.tensor_scalar_add(
                        out=ybuf[:, h, :],
                        in0=g[:, :, hcols + h],
                        scalar1=g[:, 0, hcols + heads + h : hcols + heads + h + 1],
                    )
                    nc.scalar.activation(
                        out=ebuf1[:, h, :], in_=ybuf[:, h, :],
                        func=mybir.ActivationFunctionType.Exp,
                    )
                    nc.scalar.activation(
                        out=ebuf2[:, h, :], in_=ybuf[:, h, :],
                        func=mybir.ActivationFunctionType.Exp,
                        scale=cfg.neg_slope,
                    )
                # p = max(e1, e2) == exp(leaky_relu(alpha))
                nc.vector.tensor_tensor(
                    out=pbuf[:, :, :], in0=ebuf1[:, :, :], in1=ebuf2[:, :, :],
                    op=mybir.AluOpType.max,
                )
                nc.vector.tensor_reduce(
                    out=dnm[:, :], in_=pbuf[:, :, :],
                    op=mybir.AluOpType.add, axis=mybir.AxisListType.X,
                )
                nc.vector.reciprocal(out=rcp[:, :], in_=dnm[:, :])

                # msg = h[src] * p   (c-major, k-innermost for the reduction)
                m = mbufp.tile([P, hcols, K], F32, tag="m")
                g_ap = g[:, :, :]
                gT = bass.AP(
                    tensor=g_ap.tensor,
                    offset=g_ap.offset,
                    ap=[g_ap.ap[0], [1, hcols], [d, K]],
                )
                p_ap = pbuf[:, :, :]
                p_b = bass.AP(
                    tensor=p_ap.tensor,
                    offset=p_ap.offset,
                    ap=[p_ap.ap[0], [K, heads], [0, ch], [1, K]],
                )
                nc.vector.tensor_tensor(
                    out=m[:, :, :], in0=gT, in1=p_b, op=mybir.AluOpType.mult
                )
                acc = obufp.tile([P, hcols], F32, tag="acc")
                nc.vector.tensor_reduce(
                    out=acc[:, :], in_=m[:, :, :],
                    op=mybir.AluOpType.add, axis=mybir.AxisListType.X,
                )
                o = obufp.tile([P, outw], F32, tag="o")
                for h in range(heads):
                    nc.scalar.activation(
                        out=o[:, h * ch : (h + 1) * ch],
                        in_=acc[:, h * ch : (h + 1) * ch],
                        func=(
                            mybir.ActivationFunctionType.Relu
                            if layer == 1
                            else mybir.ActivationFunctionType.Copy
                        ),
                        scale=rcp[:, h : h + 1],
                    )
                nc.sync.dma_start(out=outl[j * P : (j + 1) * P, :], in_=o[:, :])
                off += K
            assert off == cfg.sk
    nc.finalize()
    return nc


# ------------------------------------------------------------------- runner

_BUILD_CACHE: dict = {}


def _get_programs(cfg: Cfg):
    key = (cfg.npad, tuple(cfg.k_sched))
    if key not in _BUILD_CACHE:
        _BUILD_CACHE[key] = (_build_common(cfg, 1), _build_common(cfg, 2))
    return _BUILD_CACHE[key]


def _assemble(cfg: Cfg, results, width):
    g = np.zeros((cfg.npad, width), np.float32)
    for c in range(cfg.ncores):
        o = results[c]["outl"].reshape(cfg.tpc, P, width)
        for j in range(cfg.tpc):
            base = (j * cfg.ncores + c) * P
            g[base : base + P] = o[j]
    return g


def _prep_all(inputs: dict):
    cfg = Cfg()
    x = np.ascontiguousarray(np.asarray(inputs["x"], dtype=np.float32))
    perm, idx_arrays = preprocess(cfg, np.asarray(inputs["edge_index"]))
    w1e = make_wext1(
        np.asarray(inputs["W1"], np.float32),
        np.asarray(inputs["att_src1"], np.float32),
        np.asarray(inputs["att_dst1"], np.float32),
        cfg.heads, cfg.hid,
    )
    w2e = make_wext2(
        np.asarray(inputs["W2"], np.float32),
        np.asarray(inputs["att_src2"], np.float32),
        np.asarray(inputs["att_dst2"], np.float32),
        cfg.out_ch,
    )
    # biases are zero in this problem; fold anyway for safety
    b1 = np.asarray(inputs.get("b1", np.zeros(cfg.heads * cfg.hid)), np.float32)
    b2 = np.asarray(inputs.get("b2", np.zeros(cfg.out_ch)), np.float32)
    xp = np.zeros((cfg.npad, cfg.in_ch), np.float32)
    xp[: cfg.n] = x[perm]
    xt = np.ascontiguousarray(xp.T)
    return cfg, perm, idx_arrays, w1e, w2e, b1, b2, xt


def kernel(**inputs) -> np.ndarray:
    from concourse.bass_utils import run_bass_kernel_spmd

    cfg, perm, idx_arrays, w1e, w2e, b1, b2, xt = _prep_all(inputs)
    nc1, nc2 = _get_programs(cfg)
    core_ids = list(range(cfg.ncores))

    r1 = run_bass_kernel_spmd(
        nc1, [{"xt": xt, "wext": w1e, "idx": idx_arrays[c]} for c in core_ids],
        core_ids,
    )
    g1 = _assemble(cfg, r1.results, cfg.heads * cfg.hid)
    # reference applies b1 before the inter-layer relu; b1 is identically zero
    # in this problem (setup_inputs uses jnp.zeros), so the on-device relu
    # already matches. Guard against surprises:
    assert not np.any(b1), "nonzero b1 unsupported by this kernel"
    g1t = np.ascontiguousarray(g1.T)

    r2 = run_bass_kernel_spmd(
        nc2, [{"xt": g1t, "wext": w2e, "idx": idx_arrays[c]} for c in core_ids],
        core_ids,
    )
    g2 = _assemble(cfg, r2.results, cfg.out_ch)

    out = np.zeros((cfg.n, cfg.out_ch), np.float32)
    out[perm] = g2[: cfg.n]
    out += b2[None, :].astype(np.float32)  # exact: reference adds b2 last
    return out


def estimate_hw_time_ns(inputs: dict) -> int:
    """Cost-model (CoreSim clock) estimate of per-launch HW time, summed."""
    from concourse import bass_interp

    cfg, perm, idx_arrays, w1e, w2e, b1, b2, xt = _prep_all(inputs)
    nc1, nc2 = _get_programs(cfg)
    total = 0
    for nc_, wext in ((nc1, w1e), (nc2, w2e)):
        sim = bass_interp.CoreSim(nc_)
        sim.tensor("xt")[:] = np.zeros(sim.tensor("xt").shape, np.float32) if (
            nc_ is nc2
        ) else xt
        sim.tensor("wext")[:] = wext
        sim.tensor("idx")[:] = idx_arrays[0]
        sim.simulate()
        total += int(sim.time)
    return total


if __name__ == "__main__":
    # smoke run with random inputs at full size
    rng = np.random.default_rng(0)
    inputs = dict(
        x=rng.standard_normal((N_NODES, IN_CH)).astype(np.float32),
        edge_index=rng.integers(0, N_NODES, size=(2, 1600000)).astype(np.int32),
        W1=(rng.standard_normal((IN_CH, HEADS1 * HID)) / np.sqrt(IN_CH)).astype(np.float32),
        att_src1=(rng.standard_normal((HEADS1, HID)) * 0.1).astype(np.float32),
        att_dst1=(rng.standard_normal((HEADS1, HID)) * 0.1).astype(np.float32),
        b1=np.zeros(HEADS1 * HID, np.float32),
        W2=(rng.standard_normal((HEADS1 * HID, OUT_CH)) / np.sqrt(HEADS1 * HID)).astype(np.float32),
        att_src2=(rng.standard_normal((1, OUT_CH)) * 0.1).astype(np.float32),
        att_dst2=(rng.standard_normal((1, OUT_CH)) * 0.1).astype(np.float32),
        b2=np.zeros(OUT_CH, np.float32),
    )
    out = kernel(**inputs)
    print("kernel out", out.shape, out.dtype, float(np.abs(out).max()))



# revision 3
# speedup vs baseline: 1.0662x; 1.0662x over previous
"""2-layer GAT (PyG GATConv) on 8 trn2 cores — batched dma_gather design (v2).

Per layer (one shared SPMD program, per-core data):
  phase 1: replicated GEMM t_tab[pos] = x[pos] @ [W | W@att_src] in bf16
           256-B rows [msg, a_s_hi (H), a_s_lo (H), pad->128 cols], written
           2 rows/partition/descriptor (512-B descs avoid the <512B DMA
           penalty).  a_d is computed exactly on HOST (f32) and fed as a
           per-(tile,head) exp-bias array: softmax is shift-invariant so a_d
           precision only matters per-dst (exact f32 is best); a_s is
           per-edge and needs hi+lo bf16 in the table.
  phase 2: ELL edge gather via InstDMAGatherAnt.  int16 indices address only
           32768 rows, so table positions live in 4 contiguous class blocks
           of 25089 rows (25088 nodes + dummy row with a_s=-87 whose
           exp-weight underflows for padding slots).  Per chunk of tiles
           (<= CAPCOLS slot columns) 4 gather calls (one per class,
           single_packet=False) amortize the 994-ns SWDGE fixed cost; the
           994*4/chunk + 0.34/desc runs on Pool while DMA engines stream
           22.75 ns/row.  Column inflation from per-class maxima is
           minimized host-side: greedy convex 4-coloring of nodes
           (equalizes per-dst class counts) + bigness-sorted quota dealing
           of dsts into tiles + shared-across-cores schedule (~1.30x ideal).
  compute: alpha = a_s_hi+a_s_lo (DVE per class range); e1 = exp(alpha+a_d),
           e2 = exp(0.2 alpha + 0.2 a_d) (ACT, bias AP); p = max(e1,e2)
           == exp(leaky_relu(alpha+a_d)); denom reduce + reciprocal; msg
           multiply (broadcast AP) + reduce; ACT relu/copy scaled 1/denom.
"""

import os
import sys

os.environ.setdefault("JAX_PLATFORMS", "axon")
if "/opt/trn_rl_repo" not in sys.path:
    sys.path.insert(0, "/opt/trn_rl_repo")

from dataclasses import dataclass

import numpy as np

import concourse.bass as bass
import concourse.mybir as mybir
import concourse.tile as tile
from concourse import bacc

F32 = mybir.dt.float32
BF16 = mybir.dt.bfloat16
I16 = mybir.dt.int16

P = 128
ROW = 128            # bf16 elements per table row (256 B)
DUMMY_AS = -87.0

N_NODES = 100000
IN_CH = 128
HID = 32
HEADS1 = 2
OUT_CH = 32
NCORES = 8
NEG_SLOPE = 0.2

NTILES = 784         # 100352 / 128
TPC = NTILES // NCORES
BLK = NTILES * P // 4 + 1     # 25089 rows per class block (incl. dummy)
NPOS = 4 * BLK                 # 100356 table rows used
NTAB = -(-NPOS // 2048) * 2048  # 102400, GEMM-chunk padded
DUMMY_RED = BLK - 1
_OFF_MOD = int(os.environ.get("K2_OFFMOD", "3"))
_OFF_MOD_L2 = int(os.environ.get("K2_OFFMOD_L2", "1000"))
CAPCOLS = int(os.environ.get("K2_CAPCOLS", "96"))


@dataclass
class Plan:
    row_of_node: np.ndarray = None    # [npad] node -> dst row position
    node_of_row: np.ndarray = None
    ksched: np.ndarray = None         # [TPC, 4] shared per-class columns
    chunks: list = None               # chunk descriptors (shared)
    idx16: list = None                # per-core [128, NI/16] int16
    gemm_col_node: np.ndarray = None  # [NTAB] xt column -> node (-1 pad)
    ni_total: int = 0


def _color_nodes(dst_by_src, starts, outdeg, npad):
    """Greedy convex-penalty 4-coloring balancing per-dst class counts."""
    n = len(outdeg)
    pow3 = np.power(3.0, np.arange(64))
    cnt = np.zeros((n, 4), np.int64)
    cap = np.full(4, npad // 4, np.int64)
    color = np.full(n, -1, np.int8)
    proc = np.argsort(-outdeg, kind="stable")
    for node in proc:
        ds = dst_by_src[starts[node] : starts[node + 1]]
        sc = pow3[cnt[ds]].sum(0) if len(ds) else np.zeros(4)
        sc = sc + pow3[cnt[node]]
        sc[cap <= 0] = np.inf
        c = int(np.argmin(sc))
        color[node] = c
        cap[c] -= 1
        if len(ds):
            np.add.at(cnt, (ds, c), 1)  # handles duplicate (src,dst) edges
        cnt[node, c] += 1  # self loop
    return color, cnt


def preprocess(edge_index: np.ndarray) -> Plan:
    src = np.asarray(edge_index[0], dtype=np.int64)
    dst = np.asarray(edge_index[1], dtype=np.int64)
    n = N_NODES
    npad = NTILES * P

    order_e = np.argsort(src, kind="stable")
    dst_by_src = dst[order_e]
    starts = np.searchsorted(src[order_e], np.arange(n + 1))
    outdeg = np.diff(starts)

    color, cnt = _color_nodes(dst_by_src, starts, outdeg, npad)

    cap_left = npad // 4 - np.bincount(color, minlength=4)
    padcolor = np.repeat(np.arange(4), cap_left).astype(np.int64)
    allcolor = np.concatenate([color.astype(np.int64), padcolor])
    v = np.concatenate([cnt, np.zeros((npad - n, 4), np.int64)])
    v[np.arange(n, npad), padcolor] = 1  # pad-node self loop

    # table positions: class blocks
    tab_of_node = np.empty(npad, np.int64)
    for c in range(4):
        pool = np.where(allcolor == c)[0]
        assert len(pool) == npad // 4
        tab_of_node[pool] = c * BLK + np.arange(len(pool))
    red_of_node = tab_of_node - allcolor * BLK

    # dst tiles: global sort by (max comp, sum, lex), quota dealing
    M = v.max(1)
    S = v.sum(1)
    key = np.lexsort((-v[:, 3], -v[:, 2], -v[:, 1], -v[:, 0], -S, -M))
    quota = np.full((NTILES, 4), P // 4, np.int32)
    nexttile = np.zeros(4, np.int64)
    tile_of = np.empty(npad, np.int32)
    fill = np.zeros(NTILES, np.int32)
    row_of_node = np.empty(npad, np.int64)
    for node in key:
        c = allcolor[node]
        t = nexttile[c]
        while quota[t, c] == 0:
            t += 1
        nexttile[c] = t
        quota[t, c] -= 1
        tile_of[node] = t
        row_of_node[node] = t * P + fill[t]
        fill[t] += 1
    node_of_row = np.empty(npad, np.int64)
    node_of_row[row_of_node] = np.arange(npad)

    kcs = np.zeros((NTILES, 4), np.int64)
    for c in range(4):
        np.maximum.at(kcs, (tile_of, c), v[:, c])
    # shared schedule across cores: local tile j covers globals 8j..8j+7
    ksched = kcs.reshape(TPC, NCORES, 4).max(1)  # [TPC, 4]

    # per-(row, class) edge lists with tablepos-reduced indices
    e_dst = np.concatenate([dst, np.arange(npad, dtype=np.int64)])
    e_src = np.concatenate([src, np.arange(npad, dtype=np.int64)])
    e_cls = allcolor[e_src]
    e_row = row_of_node[e_dst]
    e_red = red_of_node[e_src]
    eord = np.lexsort((e_cls, e_row))
    e_row = e_row[eord]
    e_cls = e_cls[eord]
    e_red = e_red[eord]
    grp = e_row * 4 + e_cls
    gstart = np.searchsorted(grp, np.arange(npad * 4 + 1))

    # shared chunk schedule over local tiles
    chunks = []
    cur, cur_cols = [], 0
    for j in range(TPC):
        tc = int(ksched[j].sum())
        if cur and cur_cols + tc > CAPCOLS:
            chunks.append(cur)
            cur, cur_cols = [], 0
        cur.append(j)
        cur_cols += tc
    if cur:
        chunks.append(cur)

    chunk_desc = []
    for ch in chunks:
        cls_cols = []
        base_col = 0
        tdesc = []
        for c in range(4):
            ccols = 0
            for j in ch:
                kc = int(ksched[j, c])
                if kc == 0:
                    continue
                tdesc.append((j, c, base_col + ccols, kc))
                ccols += kc
            cls_cols.append(ccols)
            base_col += ccols
        chunk_desc.append(
            dict(tiles=list(ch), cls_cols=cls_cols, total=base_col, tdesc=tdesc)
        )

    # per-core int16 index arrays following the shared schedule
    idx16 = []
    for core in range(NCORES):
        vals = []
        for ch in chunk_desc:
            for c in range(4):
                for j in ch["tiles"]:
                    kc = int(ksched[j, c])
                    if kc == 0:
                        continue
                    g = j * NCORES + core
                    blk = np.full((kc, P), DUMMY_RED, np.int64)
                    for p in range(P):
                        r = g * P + p
                        s0, s1 = gstart[r * 4 + c], gstart[r * 4 + c + 1]
                        blk[: s1 - s0, p] = e_red[s0:s1]
                    vals.append(blk.reshape(-1))
        flat = np.concatenate(vals)
        assert flat.min() >= 0 and flat.max() <= DUMMY_RED
        flat = flat.astype(np.int16)
        wrapped = flat.reshape(-1, 16).T
        idx16.append(np.ascontiguousarray(np.tile(wrapped, (8, 1))))

    # GEMM column mapping: xt col q*256+s*128+p holds node at tab q*256+2p+s
    node_of_tab = np.full(NTAB, -1, np.int64)
    node_of_tab[tab_of_node] = np.arange(npad)
    ar = np.arange(NTAB)
    tabpos = (ar // 256) * 256 + 2 * (ar % 128) + (ar % 256) // 128
    gemm_col_node = node_of_tab[tabpos]

    plan = Plan()
    plan.row_of_node = row_of_node
    plan.node_of_row = node_of_row
    plan.ksched = ksched
    plan.chunks = chunk_desc
    plan.idx16 = idx16
    plan.gemm_col_node = gemm_col_node
    plan.ni_total = idx16[0].shape[1]
    return plan


# ------------------------------------------------------------- kernel builder


def build_layer(plan: Plan, layer: int):
    H = HEADS1 if layer == 1 else 1
    CH = HID if layer == 1 else OUT_CH
    MSG = H * CH
    D = MSG + H
    KIN = IN_CH if layer == 1 else HEADS1 * HID
    chunks = plan.chunks
    ni_total = plan.ni_total

    nc = bacc.Bacc(None, target_bir_lowering=False)
    xt = nc.declare_dram_parameter("xt", [KIN, NTAB], BF16, isOutput=False)
    wext = nc.declare_dram_parameter("wext", [KIN, D], BF16, isOutput=False)
    idx = nc.declare_dram_parameter("idx", [P, ni_total], I16, isOutput=False)
    adb = nc.declare_dram_parameter("adb", [P, TPC * 2 * H], F32, isOutput=False)
    outl = nc.declare_dram_parameter("outl", [TPC * P, MSG], F32, isOutput=True)
    t_tab = nc.dram_tensor("t_tab", [NTAB, ROW], BF16)

    with tile.TileContext(nc) as tc:
        with (
            tc.tile_pool(name="singles", bufs=1) as singles,
            tc.tile_pool(name="gchunk", bufs=3) as gchunk,
            tc.tile_pool(name="rows", bufs=3) as rows,
            tc.tile_pool(name="psum", bufs=4, space="PSUM") as psum,
            tc.tile_pool(name="cb", bufs=2) as cbp,
            tc.tile_pool(name="small", bufs=4) as small,
            tc.tile_pool(name="mbuf", bufs=2) as mbufp,
            tc.tile_pool(name="obuf", bufs=3) as obufp,
            tc.tile_pool(name="ibuf", bufs=2) as ibufp,
        ):
            w_s = singles.tile([KIN, D], BF16)
            nc.sync.dma_start(out=w_s[:, :], in_=wext[:, :])
            adb_s = singles.tile([P, TPC * 2 * H], F32)
            nc.sync.dma_start(out=adb_s[:, :], in_=adb[:, :])
            # dummy-row a_s hi/lo: hi = -87, lo = 0
            cw = singles.tile([4, 2 * H], BF16)
            nc.vector.memset(cw[:, 0:H], DUMMY_AS)
            nc.vector.memset(cw[:, H : 2 * H], 0.0)

            # ---- phase 1: table GEMM, 1024 positions (8 x 128 rows) per
            # chunk, 4 matmuls share one PSUM tile for batched copies.
            # a_s is stored as bf16 hi + bf16 lo residual columns.
            GC = 2048
            for q in range(NTAB // GC):
                xt_t = gchunk.tile([KIN, GC], BF16)
                nc.sync.dma_start(
                    out=xt_t[:, :], in_=xt[:, q * GC : (q + 1) * GC]
                )
                rt = rows.tile([P, GC], BF16, tag="rt")
                rt_base = rt[:, :]
                pad_ap = bass.AP(
                    tensor=rt_base.tensor,
                    offset=rt_base.offset + MSG + 2 * H,
                    ap=[rt_base.ap[0], [P, GC // P], [1, P - MSG - 2 * H]],
                )
                nc.gpsimd.memset(pad_ap, 0.0)
                for g4 in range(GC // (4 * P)):
                    ps = psum.tile([P, 4 * D], F32)
                    for si in range(4):
                        s = g4 * 4 + si
                        nc.tensor.matmul(
                            out=ps[:, si * D : (si + 1) * D],
                            lhsT=xt_t[:, s * P : (s + 1) * P],
                            rhs=w_s[:, :],
                            start=True,
                            stop=True,
                        )
                    ps_b = ps[:, :]
                    # msg + a_s hi in one bf16 copy
                    nc.scalar.activation(
                        out=bass.AP(
                            tensor=rt_base.tensor,
                            offset=rt_base.offset + g4 * 4 * P,
                            ap=[rt_base.ap[0], [P, 4], [1, D]],
                        ),
                        in_=bass.AP(
                            tensor=ps_b.tensor, offset=ps_b.offset,
                            ap=[ps_b.ap[0], [D, 4], [1, D]],
                        ),
                        func=mybir.ActivationFunctionType.Copy,
                    )
                    # a_s lo = a_s - hi
                    nc.vector.tensor_tensor(
                        out=bass.AP(
                            tensor=rt_base.tensor,
                            offset=rt_base.offset + g4 * 4 * P + D,
                            ap=[rt_base.ap[0], [P, 4], [1, H]],
                        ),
                        in0=bass.AP(
                            tensor=ps_b.tensor, offset=ps_b.offset + MSG,
                            ap=[ps_b.ap[0], [D, 4], [1, H]],
                        ),
                        in1=bass.AP(
                            tensor=rt_base.tensor,
                            offset=rt_base.offset + g4 * 4 * P + MSG,
                            ap=[rt_base.ap[0], [P, 4], [1, H]],
                        ),
                        op=mybir.AluOpType.subtract,
                    )
                # partition p holds GC//256 row-pairs: tabpos b*256 + 2p + s
                out_ap = bass.AP(
                    tensor=t_tab, offset=q * GC * ROW,
                    ap=[[256, P], [256 * P, GC // 256], [1, 256]],
                )
                in_ap2 = bass.AP(
                    tensor=rt_base.tensor, offset=rt_base.offset,
                    ap=[rt_base.ap[0], [256, GC // 256], [1, 256]],
                )
                nc.sync.dma_start(out=out_ap, in_=in_ap2)

            dummy_ap = bass.AP(
                tensor=t_tab, offset=DUMMY_RED * ROW + MSG,
                ap=[[BLK * ROW, 4], [1, 2 * H]],
            )
            nc.sync.dma_start(out=dummy_ap, in_=cw[:, :])

            tc.strict_bb_all_engine_barrier()

            # ---- phase 2
            ioff = 0
            for ch in chunks:
                total = ch["total"]
                idx_s = ibufp.tile([P, total * 8], I16, tag="ib")
                nc.sync.dma_start(
                    out=idx_s[:, :], in_=idx[:, ioff : ioff + total * 8]
                )
                iloc = 0
                cb = cbp.tile([P, total, ROW], BF16, tag="cb")
                cb_base = cb[:, :, :]
                col0 = 0
                for c in range(4):
                    ccols = ch["cls_cols"][c]
                    if ccols == 0:
                        continue
                    nidx = ccols * P
                    in_ap = bass.AP(
                        tensor=t_tab, offset=c * BLK * ROW,
                        ap=[[ROW, BLK], [1, ROW]],
                    )
                    nc.gpsimd.dma_gather(
                        out_ap=cb[:, col0 : col0 + ccols, :],
                        in_ap=in_ap,
                        idxs_ap=idx_s[:, iloc : iloc + nidx // 16],
                        num_idxs=nidx,
                        num_idxs_reg=nidx,
                        elem_size=ROW,
                        single_packet=False,
                    )
                    iloc += nidx // 16
                    ioff += nidx // 16
                    col0 += ccols
                assert col0 == total

                for j in ch["tiles"]:
                    ranges = [t for t in ch["tdesc"] if t[0] == j]
                    kp = int(plan.ksched[j].sum())
                    if kp == 0:
                        continue
                    ebuf = small.tile([P, H, kp], F32, tag="e")
                    e1 = small.tile([P, H, kp], BF16, tag="e1")
                    e2 = small.tile([P, H, kp], BF16, tag="e2")
                    pb = small.tile([P, H, kp], BF16, tag="p")
                    dnm = small.tile([P, H], F32, tag="d")
                    rcp = small.tile([P, H], F32, tag="r")
                    m = mbufp.tile([P, MSG, kp], BF16, tag="m")
                    eb_base = ebuf[:, :, :]
                    pb_base = pb[:, :, :]
                    m_base = m[:, :, :]

                    # alpha(a_s) = hi + lo into contiguous f32 ebuf
                    toff = 0
                    for (_, c, cst, kc) in ranges:
                        hi = bass.AP(
                            tensor=cb_base.tensor,
                            offset=cb_base.offset + cst * ROW + MSG,
                            ap=[cb_base.ap[0], [1, H], [ROW, kc]],
                        )
                        lo = bass.AP(
                            tensor=cb_base.tensor,
                            offset=cb_base.offset + cst * ROW + MSG + H,
                            ap=[cb_base.ap[0], [1, H], [ROW, kc]],
                        )
                        eb = bass.AP(
                            tensor=eb_base.tensor,
                            offset=eb_base.offset + toff,
                            ap=[eb_base.ap[0], [kp, H], [1, kc]],
                        )
                        nc.vector.tensor_tensor(
                            out=eb, in0=hi, in1=lo, op=mybir.AluOpType.add
                        )
                        toff += kc
                    assert toff == kp

                    for h in range(H):
                        nc.scalar.activation(
                            out=e1[:, h, :], in_=ebuf[:, h, :],
                            func=mybir.ActivationFunctionType.Exp,
                            bias=adb_s[:, j * 2 * H + h : j * 2 * H + h + 1],
                        )
                        nc.scalar.activation(
                            out=e2[:, h, :], in_=ebuf[:, h, :],
                            func=mybir.ActivationFunctionType.Exp,
                            scale=NEG_SLOPE,
                            bias=adb_s[
                                :, j * 2 * H + H + h : j * 2 * H + H + h + 1
                            ],
                        )
                    nc.vector.tensor_tensor(
                        out=pb[:, :, :], in0=e1[:, :, :], in1=e2[:, :, :],
                        op=mybir.AluOpType.max,
                    )
                    nc.vector.tensor_reduce(
                        out=dnm[:, :], in_=pb[:, :, :],
                        op=mybir.AluOpType.add, axis=mybir.AxisListType.X,
                    )
                    nc.vector.reciprocal(out=rcp[:, :], in_=dnm[:, :])

                    toff = 0
                    for (_, c, cst, kc) in ranges:
                        g_in = bass.AP(
                            tensor=cb_base.tensor,
                            offset=cb_base.offset + cst * ROW,
                            ap=[cb_base.ap[0], [CH, H], [1, CH], [ROW, kc]],
                        )
                        p_in = bass.AP(
                            tensor=pb_base.tensor,
                            offset=pb_base.offset + toff,
                            ap=[pb_base.ap[0], [kp, H], [0, CH], [1, kc]],
                        )
                        m_out = bass.AP(
                            tensor=m_base.tensor,
                            offset=m_base.offset + toff,
                            ap=[m_base.ap[0], [CH * kp, H], [kp, CH], [1, kc]],
                        )
                        off_mod = _OFF_MOD if layer == 1 else _OFF_MOD_L2
                        mul_eng = nc.gpsimd if (j % off_mod == off_mod - 1) else nc.vector
                        mul_eng.tensor_tensor(
                            out=m_out, in0=g_in, in1=p_in,
                            op=mybir.AluOpType.mult,
                        )
                        toff += kc

                    # in-place halving tree sum over k (bf16 2x mode)
                    w = kp
                    while w > 1:
                        a = w // 2
                        left = bass.AP(
                            tensor=m_base.tensor, offset=m_base.offset,
                            ap=[m_base.ap[0], [kp, MSG], [1, a]],
                        )
                        right = bass.AP(
                            tensor=m_base.tensor,
                            offset=m_base.offset + (w - a),
                            ap=[m_base.ap[0], [kp, MSG], [1, a]],
                        )
                        with nc.allow_low_precision(
                            reason="bf16 msg-sum validated at 5e-3 rel err"
                        ):
                            nc.vector.tensor_tensor(
                                out=left, in0=left, in1=right,
                                op=mybir.AluOpType.add,
                            )
                        w -= a
                    o = obufp.tile([P, MSG], F32, tag="o")
                    for h in range(H):
                        nc.scalar.activation(
                            out=o[:, h * CH : (h + 1) * CH],
                            in_=bass.AP(
                                tensor=m_base.tensor,
                                offset=m_base.offset + h * CH * kp,
                                ap=[m_base.ap[0], [kp, CH]],
                            ),
                            func=(
                                mybir.ActivationFunctionType.Relu
                                if layer == 1
                                else mybir.ActivationFunctionType.Copy
                            ),
                            scale=rcp[:, h : h + 1],
                        )
                    nc.sync.dma_start(
                        out=outl[j * P : (j + 1) * P, :], in_=o[:, :]
                    )
            assert ioff == ni_total
    nc.finalize()
    return nc


# ------------------------------------------------------------------- runner


def _to_bf16(x):
    import ml_dtypes

    return np.asarray(x).astype(ml_dtypes.bfloat16)


def _host_tab_inputs(plan: Plan, xfull, W, att_src, att_dst, H, CH):
    KIN = xfull.shape[1]
    MSG = H * CH
    wext = np.zeros((KIN, MSG + H), np.float32)
    wext[:, :MSG] = W
    for h in range(H):
        wext[:, MSG + h] = W[:, h * CH : (h + 1) * CH] @ att_src[h]

    npad = NTILES * P
    xp = np.zeros((npad, KIN), np.float32)
    xp[:N_NODES] = xfull
    xt = np.zeros((NTAB, KIN), np.float32)
    valid = plan.gemm_col_node >= 0
    xt[valid] = xp[plan.gemm_col_node[valid]]
    xt_bf = _to_bf16(np.ascontiguousarray(xt.T))

    ad = ((xp @ W).reshape(npad, H, CH) * att_dst[None]).sum(-1)  # [npad, H]
    return xt_bf, _to_bf16(wext), ad.astype(np.float32)


def _adb_for_core(plan, ad, core, H):
    adb = np.zeros((P, TPC * 2 * H), np.float32)
    for j in range(TPC):
        g = j * NCORES + core
        nodes = plan.node_of_row[g * P : (g + 1) * P]
        a = ad[nodes]
        adb[:, j * 2 * H : j * 2 * H + H] = a
        adb[:, j * 2 * H + H : j * 2 * H + 2 * H] = NEG_SLOPE * a
    return adb


_BUILD_CACHE = {}


def _get_program(plan: Plan, layer: int):
    if layer not in _BUILD_CACHE:
        _BUILD_CACHE[layer] = build_layer(plan, layer)
    return _BUILD_CACHE[layer]


def _assemble(plan: Plan, results, width):
    g = np.zeros((NTILES * P, width), np.float32)
    for c in range(NCORES):
        o = results[c]["outl"].reshape(TPC, P, width)
        for j in range(TPC):
            gt = j * NCORES + c
            g[gt * P : (gt + 1) * P] = o[j]
    return g


def kernel(**inputs) -> np.ndarray:
    from concourse.bass_utils import run_bass_kernel_spmd

    x = np.asarray(inputs["x"], np.float32)
    plan = preprocess(np.asarray(inputs["edge_index"]))
    W1 = np.asarray(inputs["W1"], np.float32)
    as1 = np.asarray(inputs["att_src1"], np.float32)
    ad1 = np.asarray(inputs["att_dst1"], np.float32)
    W2 = np.asarray(inputs["W2"], np.float32)
    as2 = np.asarray(inputs["att_src2"], np.float32)
    ad2 = np.asarray(inputs["att_dst2"], np.float32)
    b2 = np.asarray(inputs.get("b2", np.zeros(OUT_CH)), np.float32)
    assert not np.any(np.asarray(inputs.get("b1", 0.0))), "b1 must be zero"

    core_ids = list(range(NCORES))

    xt1, w1, adarr1 = _host_tab_inputs(plan, x, W1, as1, ad1, HEADS1, HID)
    prog1 = _get_program(plan, 1)
    feeds1 = [
        {"xt": xt1, "wext": w1, "idx": plan.idx16[c],
         "adb": _adb_for_core(plan, adarr1, c, HEADS1)}
        for c in core_ids
    ]
    r1 = run_bass_kernel_spmd(prog1, feeds1, core_ids)
    g1 = _assemble(plan, r1.results, HEADS1 * HID)  # post-relu, row order

    h1 = g1[plan.row_of_node]  # node order
    xt2, w2, adarr2 = _host_tab_inputs(
        plan, h1[:N_NODES], W2, as2, ad2, 1, OUT_CH
    )
    prog2 = _get_program(plan, 2)
    feeds2 = [
        {"xt": xt2, "wext": w2, "idx": plan.idx16[c],
         "adb": _adb_for_core(plan, adarr2, c, 1)}
        for c in core_ids
    ]
    r2 = run_bass_kernel_spmd(prog2, feeds2, core_ids)
    g2 = _assemble(plan, r2.results, OUT_CH)

    out = g2[plan.row_of_node][:N_NODES] + b2[None, :]
    return out.astype(np.float32)


def estimate_hw_time_ns(inputs: dict) -> int:
    from concourse import bass_interp

    x = np.asarray(inputs["x"], np.float32)
    plan = preprocess(np.asarray(inputs["edge_index"]))
    W1 = np.asarray(inputs["W1"], np.float32)
    as1 = np.asarray(inputs["att_src1"], np.float32)
    ad1 = np.asarray(inputs["att_dst1"], np.float32)
    xt1, w1, adarr1 = _host_tab_inputs(plan, x, W1, as1, ad1, HEADS1, HID)
    total = 0
    for layer in (1, 2):
        prog = _get_program(plan, layer)
        sim = bass_interp.CoreSim(prog)
        if layer == 1:
            sim.tensor("xt")[:] = xt1
            sim.tensor("wext")[:] = w1
            sim.tensor("adb")[:] = _adb_for_core(plan, adarr1, 0, HEADS1)
        else:
            sim.tensor("xt")[:] = np.zeros(
                sim.tensor("xt").shape, sim.tensor("xt").dtype
            )
            sim.tensor("wext")[:] = np.zeros(
                sim.tensor("wext").shape, sim.tensor("wext").dtype
            )
            sim.tensor("adb")[:] = np.ones(sim.tensor("adb").shape, np.float32)
        sim.tensor("idx")[:] = plan.idx16[0]
        sim.simulate()
        total += int(sim.time)
    return total
